# revision 26
# baseline (speedup 1.0000x reference)
"""Trainium2 Bass kernel for nn_BDFM_Multi (B=8,C=256,H=W=128,N=4).

Data-parallel over batch: one batch element per NeuronCore (8 cores).

Per-core computation (feature f [C,HW], m [N,H,W], HW=16384):
  z    = (m > 0.3); er/dl = 13-tap separable binary morphology
  fbu  = per-class (er, 1-dl, dl-er) 0/1 masks        [12, HW]
  mid  = fbu @ f^T                                    [12, C]
  A'   = Wo2' @ mid^T ; G = A' @ mid                  [C, C]
  Wc   = Wo1' + G @ Wf'                               [C, C]
  out  = Wc @ f + u,  u = G @ beta_f + beta_o         [C, HW]

Split of work:
  - host: morphology masks (exact integer band-matmul method), bf16 casts,
    both f layouts (natural + transposed), weight folding
  - device: mid accumulation over pre-transposed f^T chunks, the small
    G/Wc algebra, and the big pass-2 stream out = Wc@f + u, all bf16
    streams with fp32 PSUM accumulation; out stored bf16, host upcasts
"""
import numpy as np
import ml_dtypes
from contextlib import ExitStack

import concourse.bass as bass
import concourse.mybir as mybir
import concourse.tile as tile
from concourse import bacc
from concourse import bass_utils

F32 = mybir.dt.float32
BF16 = mybir.dt.bfloat16
ALU = mybir.AluOpType
ACTF = mybir.ActivationFunctionType

B, C, H, W, N = 8, 256, 128, 128, 4
HW = H * W
EPS = 1e-5
P = 128
PT = 512              # p-tile width for pass 2
NPT = HW // PT        # 32 p-tiles

_NC_CACHE = {}

# pkb (bf16) [128, 1040]:
#   [0:512)     wo2t  : Wo2'^T chunks  [ci -> cols ci*256:(ci+1)*256]
#   [512:1024)  wf_n  : Wf' blocks     [(ci*2+a)*128 ...] = Wf'[ci*128:,a*128:]
#   [1024:1026) beta_f per ci chunk
#   [1026:1038) identity12 on partitions 0:12
#   [1038:1040) pad
PKB_W = 1040
# pkf (f32) [128, 514]:
#   [0:512) wo1t blocks (ci*2+o)   [512:514) beta_o
PKF_W = 514
# erdl (bf16) [128, 1536]: fbu channels [w, h*12+k], k = 3n/3n+1/3n+2 =
#   er_n / bg_n / maskd_n (all 0/1 masks, exact in bf16); the chunk-h
#   stationary erdl[:, h*12:h*12+12] is contiguous


def build():
    if "nc" in _NC_CACHE:
        return _NC_CACHE["nc"]
    nc = bacc.Bacc(trn_type="TRN2", target_bir_lowering=False, debug=False)

    featn = nc.dram_tensor("featn", [P, 2 * HW], BF16, kind="ExternalInput")
    featt = nc.dram_tensor("featt", [P, 2 * HW], BF16, kind="ExternalInput")
    pkb = nc.dram_tensor("pkb", [P, PKB_W], BF16, kind="ExternalInput")
    pkf = nc.dram_tensor("pkf", [P, PKF_W], F32, kind="ExternalInput")
    erdl_d = nc.dram_tensor("erdl", [P, P * 12], BF16, kind="ExternalInput")
    out = nc.dram_tensor("out", [P, 2 * HW], BF16, kind="ExternalOutput")

    with tile.TileContext(nc) as tc, ExitStack() as ctx:
        persist = ctx.enter_context(tc.tile_pool(name="persist", bufs=1))

        # ---- loads: small params first on sync; erdl on gpsimd queue ----
        pkf_sb = persist.tile([P, PKF_W], F32)
        nc.sync.dma_start(out=pkf_sb[:], in_=pkf[:])
        wo1_sb = pkf_sb[:, 0:512]
        betao_sb = pkf_sb[:, 512:514]

        pkb_sb = persist.tile([P, PKB_W], BF16)
        nc.sync.dma_start(out=pkb_sb[:], in_=pkb[:])
        wo2_sb = pkb_sb[:, 0:512]
        wfn_sb = pkb_sb[:, 512:1024]
        betaf_sb = pkb_sb[:, 1024:1026]
        ident12 = pkb_sb[0:12, 1026:1038]

        erdl = persist.tile([P, P * 12], BF16)
        nc.gpsimd.dma_start(out=erdl[:], in_=erdl_d[:])

        # f^T next on sync: mid trails this load
        ft_sb = persist.tile([P, 2 * HW], BF16)   # [w, h*256 + c]
        for q in range(8):
            nc.sync.dma_start(out=ft_sb[:, q * 4096:(q + 1) * 4096],
                              in_=featt[:, q * 4096:(q + 1) * 4096])

        # f natural second: pass 2 consumes it, blk-interleaved per 2048 cols
        fn_sb = persist.tile([P, 2 * HW], BF16)   # c-blk0 | c-blk1
        for q in range(8):
            for blk in range(2):
                nc.sync.dma_start(
                    out=fn_sb[:, blk * HW + q * 2048: blk * HW + (q + 1) * 2048],
                    in_=featn[:, blk * HW + q * 2048: blk * HW + (q + 1) * 2048])

        wc_sb = persist.tile([P, 512], BF16)       # Wc^T blocks (a*2+o)
        u_sb = persist.tile([P, 2], F32)           # bias per o-blk
        wsrc = persist.tile([P, 512], BF16)
        zerob = persist.tile([P, 1], F32)
        actwarm = persist.tile([P, 1], F32)
        mid_b = persist.tile([12, 256], BF16)

        # ---- PE p-state ramp + scalar act-table preload ----
        with tc.tile_pool(name="warm_ps", bufs=1, space="PSUM") as wps:
            nc.vector.memset(wsrc[:], 0.0)
            nc.vector.memset(zerob[:], 0.0)
            wp = wps.tile([P, 512], F32)
            for i in range(6):
                nc.tensor.matmul(wp[:], wsrc[:, 0:P], wsrc[:],
                                 start=True, stop=True, skip_group_check=True)
            nc.scalar.activation(actwarm[:], wp[:, 0:1], ACTF.Identity,
                                 bias=zerob[:])

        # ---------------- pass 1: mid = fbu @ f^T --------------
        # 4 interleaved PSUM accumulators so consecutive matmuls hit
        # different banks; reduce ends directly in bf16 mid_b.
        t0 = persist.tile([12, 512], F32)
        with tc.tile_pool(name="mid_ps", bufs=1, space="PSUM") as midps:
            mps = [midps.tile([12, 256], F32, name=f"midacc{i}")
                   for i in range(4)]
            for h in range(P):
                nc.tensor.matmul(mps[h % 4][:],
                                 erdl[:, h * 12:h * 12 + 12],
                                 ft_sb[:, h * 256:h * 256 + 256],
                                 start=(h < 4), stop=(h >= P - 4),
                                 skip_group_check=True)
            nc.vector.tensor_copy(t0[:, 0:256], mps[0][:])
            nc.vector.tensor_tensor(t0[:, 256:512], mps[1][:], t0[:, 0:256],
                                    op=ALU.add)
            nc.vector.tensor_tensor(t0[:, 0:256], mps[2][:], t0[:, 256:512],
                                    op=ALU.add)
            nc.vector.tensor_tensor(mid_b[:], mps[3][:], t0[:, 0:256],
                                    op=ALU.add)

        # ---------------- small stage: mid^T, A'^T, G^T, u, Wc -------
        with tc.tile_pool(name="sm_ps", bufs=1, space="PSUM") as smps, \
             tc.tile_pool(name="sm_sb", bufs=1) as smsb:
            # mid^T via PE transpose of [12,128] chunks
            ps_mt = smps.tile([P, 24], BF16, tag="mt")
            for ci in range(2):
                nc.tensor.matmul(ps_mt[:, ci * 12:(ci + 1) * 12],
                                 mid_b[:, ci * P:(ci + 1) * P],
                                 ident12, is_transpose=True)
            mid_t = smsb.tile([P, 24], BF16)
            nc.vector.tensor_copy(mid_t[:], ps_mt[:])

            # A'^T = mid @ Wo2'^T   [12, 256]
            ps_at = smps.tile([12, 256], F32, tag="at")
            nc.tensor.matmul(ps_at[:], mid_t[:, 0:12], wo2_sb[:, 0:256],
                             start=True, stop=False)
            nc.tensor.matmul(ps_at[:], mid_t[:, 12:24], wo2_sb[:, 256:512],
                             start=False, stop=True)
            a_t = smsb.tile([12, 256], BF16)
            nc.vector.tensor_copy(a_t[:], ps_at[:])

            # G^T[c, o] = sum_k mid[k, c] A'^T[k, o];  chunks ci on partitions
            ps_gt = smps.tile([P, 512], F32, tag="gt")
            for ci in range(2):
                nc.tensor.matmul(ps_gt[:, ci * 256:(ci + 1) * 256],
                                 mid_b[:, ci * P:(ci + 1) * P], a_t[:],
                                 start=True, stop=True)
            gt_r = smsb.tile([P, 512], BF16)
            nc.vector.tensor_copy(gt_r[:], ps_gt[:])

            # u = G @ beta_f + beta_o  (before X so the act ops overlap X)
            for o in range(2):
                ps_u = smps.tile([P, 1], F32, tag="u")
                nc.tensor.matmul(ps_u[:], gt_r[:, o * P:(o + 1) * P],
                                 betaf_sb[:, 0:1], start=True, stop=False)
                nc.tensor.matmul(ps_u[:], gt_r[:, 256 + o * P:256 + (o + 1) * P],
                                 betaf_sb[:, 1:2], start=False, stop=True)
                nc.scalar.activation(u_sb[:, o:o + 1], ps_u[:], ACTF.Identity,
                                     bias=betao_sb[:, o:o + 1])

            # X = Wf'^T @ G^T (= (G Wf')^T); blocks a (c_in chunk) on partitions
            ps_x = smps.tile([P, 512], F32, tag="x")
            for a in range(2):
                for ci in range(2):
                    nc.tensor.matmul(ps_x[:, a * 256:(a + 1) * 256],
                                     wfn_sb[:, (ci * 2 + a) * P:(ci * 2 + a + 1) * P],
                                     gt_r[:, ci * 256:(ci + 1) * 256],
                                     start=(ci == 0), stop=(ci == 1),
                                     skip_group_check=True)
            # Wc^T = Wo1'^T + X  (blocks (a*2+o) align with [a*256 + o*128])
            for a in range(2):
                nc.vector.tensor_tensor(wc_sb[:, a * 256:(a + 1) * 256],
                                        ps_x[:, a * 256:(a + 1) * 256],
                                        wo1_sb[:, a * 256:(a + 1) * 256],
                                        op=ALU.add)

        # ---------------- pass 2: out = Wc @ f + u ----------------
        # stores go out per 2 tiles (1024 cols) so they stream continuously
        with tc.tile_pool(name="out_ps", bufs=4, space="PSUM") as outps, \
             tc.tile_pool(name="p2_sb", bufs=2) as p2sb:
            for tg in range(NPT // 4):
                ot0 = p2sb.tile([P, 4 * PT], BF16, tag="ot0")
                ot1 = p2sb.tile([P, 4 * PT], BF16, tag="ot1")
                for tt in range(4):
                    t = tg * 4 + tt
                    c0 = t * PT
                    out_ps = outps.tile([P, 2 * PT], F32, tag="ops")
                    for a in range(2):
                        for o in range(2):
                            nc.tensor.matmul(
                                out_ps[:, o * PT:(o + 1) * PT],
                                wc_sb[:, (a * 2 + o) * P:(a * 2 + o + 1) * P],
                                fn_sb[:, a * HW + c0:a * HW + c0 + PT],
                                start=(a == 0), stop=(a == 1),
                                skip_group_check=True)
                    nc.scalar.activation(ot0[:, tt * PT:(tt + 1) * PT],
                                         out_ps[:, 0:PT],
                                         ACTF.Identity, bias=u_sb[:, 0:1])
                    nc.vector.tensor_scalar(ot1[:, tt * PT:(tt + 1) * PT],
                                            out_ps[:, PT:2 * PT],
                                            u_sb[:, 1:2], None, op0=ALU.add)
                g0 = tg * 4 * PT
                if tg < NPT // 4 - 1:
                    nc.gpsimd.dma_start(out=out[:, g0:g0 + 4 * PT], in_=ot0[:])
                    nc.gpsimd.dma_start(out=out[:, HW + g0:HW + g0 + 4 * PT],
                                        in_=ot1[:])
                else:
                    # split the last group's stores so the tail is short
                    for hh in range(2):
                        s0 = hh * 2 * PT
                        nc.gpsimd.dma_start(
                            out=out[:, g0 + s0:g0 + s0 + 2 * PT],
                            in_=ot0[:, s0:s0 + 2 * PT])
                        nc.gpsimd.dma_start(
                            out=out[:, HW + g0 + s0:HW + g0 + s0 + 2 * PT],
                            in_=ot1[:, s0:s0 + 2 * PT])

    nc.compile()
    _NC_CACHE["nc"] = nc
    return nc


def _band_consts():
    idx = np.arange(P)
    # erosion: output i covers input [i-8, i+4]; dilation: [i-4, i+8]
    band_er = ((idx[:, None] >= idx[None, :] - 8) &
               (idx[:, None] <= idx[None, :] + 4)).astype(np.float32)
    band_dl = ((idx[:, None] >= idx[None, :] - 4) &
               (idx[:, None] <= idx[None, :] + 8)).astype(np.float32)
    return band_er, band_dl


def _host_masks(mb):
    """fbu channels for one batch, transposed: [w, k*128+h] with k = 3n
    (er_n), 3n+1 (bg_n), 3n+2 (maskd_n) -- all 0/1, exact in bf16.

    Exact integer morphology via the band-matmul method (composition of 4
    cv2-style 4x4 erode/dilate iterations = 13-tap separable min/max)."""
    band_er, band_dl = _band_consts()
    cnt = band_er.sum(axis=0)                      # [128] per output index
    z = (mb > 0.3).astype(np.float32)              # [N, H, W]
    # rows pass: out[i, n, w] = sum_h band[h, i] z[n, h, w]
    r_er = (np.einsum('hi,nhw->inw', band_er, z)
            == cnt[:, None, None]).astype(np.float32)
    r_dl = (np.einsum('hi,nhw->inw', band_dl, z) > 0.5).astype(np.float32)
    # cols pass: er_t[iw, n, ih] = sum_w band[w, iw] rows[ih, n, w]
    er_t = np.einsum('wi,hnw->inh', band_er, r_er) == cnt[:, None, None]
    dl_t = np.einsum('wi,hnw->inh', band_dl, r_dl) > 0.5
    fbu = np.empty((P, P, 12), np.float32)         # [w, h, k]
    for n in range(N):
        fbu[:, :, 3 * n] = er_t[:, n, :]
        fbu[:, :, 3 * n + 1] = ~dl_t[:, n, :]
        fbu[:, :, 3 * n + 2] = dl_t[:, n, :] & ~er_t[:, n, :]
    return fbu.reshape(P, 12 * P).astype(ml_dtypes.bfloat16)


def prepare_in_maps(feature, m, W_f, g_f, b_f, mu_f, v_f, W_o, g_o, b_o, mu_o, v_o):
    feature = np.asarray(feature, dtype=np.float32)
    m = np.asarray(m, dtype=np.float32)
    W_f = np.asarray(W_f, dtype=np.float32)
    W_o = np.asarray(W_o, dtype=np.float32)
    g_f, b_f, mu_f, v_f = (np.asarray(x, dtype=np.float32) for x in (g_f, b_f, mu_f, v_f))
    g_o, b_o, mu_o, v_o = (np.asarray(x, dtype=np.float32) for x in (g_o, b_o, mu_o, v_o))

    inv_f = g_f / np.sqrt(v_f + EPS)
    beta_f_v = b_f - mu_f * inv_f
    inv_o = g_o / np.sqrt(v_o + EPS)
    beta_o_v = b_o - mu_o * inv_o
    Wf_p = (inv_f[:, None] * W_f).astype(np.float32)          # [C, C]
    Wo1_p = (inv_o[:, None] * W_o[:, :C]).astype(np.float32)  # [C, C]
    Wo2_p = (inv_o[:, None] * W_o[:, C:]).astype(np.float32)  # [C, C]

    def blocks_t(Wp):
        # lhsT layout: blocks ci*2+o of Wp^T
        a = np.empty((P, 512), np.float32)
        for ci in range(2):
            for o in range(2):
                a[:, (ci * 2 + o) * P:(ci * 2 + o + 1) * P] = \
                    Wp[o * P:(o + 1) * P, ci * P:(ci + 1) * P].T
        return a

    def blocks_n(Wp):
        # natural-layout blocks ci*2+a: Wp[ci*128:(ci+1)*128, a*128:(a+1)*128]
        a_ = np.empty((P, 512), np.float32)
        for ci in range(2):
            for a in range(2):
                a_[:, (ci * 2 + a) * P:(ci * 2 + a + 1) * P] = \
                    Wp[ci * P:(ci + 1) * P, a * P:(a + 1) * P]
        return a_

    bf = ml_dtypes.bfloat16

    pkb = np.zeros((P, PKB_W), bf)
    pkb[:, 0:512] = np.concatenate([Wo2_p.T[0:P, :], Wo2_p.T[P:C, :]],
                                   axis=1).astype(bf)
    pkb[:, 512:1024] = blocks_n(Wf_p).astype(bf)
    pkb[:, 1024:1026] = beta_f_v.reshape(2, P).T.astype(bf)
    pkb[0:12, 1026:1038] = np.eye(12, dtype=np.float32).astype(bf)

    pkf = np.zeros((P, PKF_W), np.float32)
    pkf[:, 0:512] = blocks_t(Wo1_p)
    pkf[:, 512:514] = beta_o_v.reshape(2, P).T

    in_maps = []
    for b in range(B):
        im = {"pkb": pkb, "pkf": pkf}
        im["erdl"] = _host_masks(m[b])
        f16 = feature[b].reshape(C, HW).astype(bf)
        im["featn"] = np.ascontiguousarray(
            np.concatenate([f16[0:P, :], f16[P:C, :]], axis=1))
        # featt[w, h*256 + c] = f[c, h*128 + w]
        im["featt"] = np.ascontiguousarray(
            f16.reshape(C, P, P).transpose(2, 1, 0).reshape(P, 2 * HW))
        in_maps.append(im)
    return in_maps


def kernel(feature, m, W_f, g_f, b_f, mu_f, v_f, W_o, g_o, b_o, mu_o, v_o):
    nc = build()
    in_maps = prepare_in_maps(feature, m, W_f, g_f, b_f, mu_f, v_f,
                              W_o, g_o, b_o, mu_o, v_o)
    res = bass_utils.run_bass_kernel_spmd(nc, in_maps, list(range(B)))
    out = np.empty((B, C, H, W), np.float32)
    for b in range(B):
        o = np.asarray(res.results[b]["out"]).astype(np.float32)
        out[b, 0:P] = o[:, 0:HW].reshape(P, H, W)
        out[b, P:C] = o[:, HW:2 * HW].reshape(P, H, W)
    return out


# revision 27
# speedup vs baseline: 1.1767x; 1.1767x over previous
"""Trainium2 Bass kernel for nn_BDFM_Multi (B=8,C=256,H=W=128,N=4).

Data-parallel over batch: one batch element per NeuronCore (8 cores).

Per-core computation (feature f [C,HW], m [N,H,W], HW=16384):
  z    = (m > 0.3); er/dl = 13-tap separable binary morphology
  fbu  = per-class (er, 1-dl, dl-er) 0/1 masks        [12, HW]
  mid  = fbu @ f^T                                    [12, C]
  A'   = Wo2' @ mid^T ; G = A' @ mid                  [C, C]
  Wc   = Wo1' + G @ Wf'                               [C, C]
  out  = Wc @ f + u,  u = G @ beta_f + beta_o         [C, HW]

Split of work:
  - host: morphology masks (exact integer band-matmul method), bf16 casts,
    both f layouts (natural + transposed), weight folding
  - device: mid accumulation over pre-transposed f^T chunks, the small
    G/Wc algebra, and the big pass-2 stream out = Wc@f + u, all bf16
    streams with fp32 PSUM accumulation; out stored bf16, host upcasts
"""
import numpy as np
import ml_dtypes
from contextlib import ExitStack

import concourse.bass as bass
import concourse.mybir as mybir
import concourse.tile as tile
from concourse import bacc
from concourse import bass_utils

F32 = mybir.dt.float32
BF16 = mybir.dt.bfloat16
ALU = mybir.AluOpType
ACTF = mybir.ActivationFunctionType

B, C, H, W, N = 8, 256, 128, 128, 4
HW = H * W
EPS = 1e-5
P = 128
PT = 512              # p-tile width for pass 2
NPT = HW // PT        # 32 p-tiles

_NC_CACHE = {}

# pkb (bf16) [128, 1040]:
#   [0:512)     wo2t  : Wo2'^T chunks  [ci -> cols ci*256:(ci+1)*256]
#   [512:1024)  wf_n  : Wf' blocks     [(ci*2+a)*128 ...] = Wf'[ci*128:,a*128:]
#   [1024:1026) beta_f per ci chunk
#   [1026:1038) identity12 on partitions 0:12
#   [1038:1040) pad
PKB_W = 1040
# pkf (f32) [128, 514]:
#   [0:512) wo1t blocks (ci*2+o)   [512:514) beta_o
PKF_W = 514
# erdl (bf16) [128, 1536]: fbu channels [w, h*12+k], k = 3n/3n+1/3n+2 =
#   er_n / bg_n / maskd_n (all 0/1 masks, exact in bf16); the chunk-h
#   stationary erdl[:, h*12:h*12+12] is contiguous


def build():
    if "nc" in _NC_CACHE:
        return _NC_CACHE["nc"]
    nc = bacc.Bacc(trn_type="TRN2", target_bir_lowering=False, debug=False)

    featn = nc.dram_tensor("featn", [P, 2 * HW], BF16, kind="ExternalInput")
    featt = nc.dram_tensor("featt", [P, 2 * HW], BF16, kind="ExternalInput")
    pkb = nc.dram_tensor("pkb", [P, PKB_W], BF16, kind="ExternalInput")
    pkf = nc.dram_tensor("pkf", [P, PKF_W], F32, kind="ExternalInput")
    erdl_d = nc.dram_tensor("erdl", [P, P * 12], BF16, kind="ExternalInput")
    out = nc.dram_tensor("out", [P, 2 * HW], BF16, kind="ExternalOutput")

    with tile.TileContext(nc) as tc, ExitStack() as ctx:
        persist = ctx.enter_context(tc.tile_pool(name="persist", bufs=1))

        # ---- loads: small params first on sync; erdl on gpsimd queue ----
        pkf_sb = persist.tile([P, PKF_W], F32)
        nc.sync.dma_start(out=pkf_sb[:], in_=pkf[:])
        wo1_sb = pkf_sb[:, 0:512]
        betao_sb = pkf_sb[:, 512:514]

        pkb_sb = persist.tile([P, PKB_W], BF16)
        nc.sync.dma_start(out=pkb_sb[:], in_=pkb[:])
        wo2_sb = pkb_sb[:, 0:512]
        wfn_sb = pkb_sb[:, 512:1024]
        betaf_sb = pkb_sb[:, 1024:1026]
        ident12 = pkb_sb[0:12, 1026:1038]

        erdl = persist.tile([P, P * 12], BF16)
        nc.gpsimd.dma_start(out=erdl[:], in_=erdl_d[:])

        # f^T next on sync: mid trails this load
        ft_sb = persist.tile([P, 2 * HW], BF16)   # [w, h*256 + c]
        for q in range(8):
            nc.sync.dma_start(out=ft_sb[:, q * 4096:(q + 1) * 4096],
                              in_=featt[:, q * 4096:(q + 1) * 4096])

        # f natural second: pass 2 consumes it, blk-interleaved per 2048 cols
        fn_sb = persist.tile([P, 2 * HW], BF16)   # c-blk0 | c-blk1
        for q in range(8):
            for blk in range(2):
                nc.sync.dma_start(
                    out=fn_sb[:, blk * HW + q * 2048: blk * HW + (q + 1) * 2048],
                    in_=featn[:, blk * HW + q * 2048: blk * HW + (q + 1) * 2048])

        wc_sb = persist.tile([P, 512], BF16)       # Wc^T blocks (a*2+o)
        u_sb = persist.tile([P, 2], F32)           # bias per o-blk
        wsrc = persist.tile([P, 512], BF16)
        zerob = persist.tile([P, 1], F32)
        actwarm = persist.tile([P, 1], F32)
        mid_b = persist.tile([12, 256], BF16)

        # ---- PE p-state ramp + scalar act-table preload ----
        with tc.tile_pool(name="warm_ps", bufs=1, space="PSUM") as wps:
            nc.vector.memset(wsrc[:], 0.0)
            nc.vector.memset(zerob[:], 0.0)
            wp = wps.tile([P, 512], F32)
            for i in range(6):
                nc.tensor.matmul(wp[:], wsrc[:, 0:P], wsrc[:],
                                 start=True, stop=True, skip_group_check=True)
            nc.scalar.activation(actwarm[:], wp[:, 0:1], ACTF.Identity,
                                 bias=zerob[:])

        # ---------------- pass 1: mid = fbu @ f^T --------------
        # 4 interleaved PSUM accumulators so consecutive matmuls hit
        # different banks; reduce ends directly in bf16 mid_b.
        t0 = persist.tile([12, 512], F32)
        with tc.tile_pool(name="mid_ps", bufs=1, space="PSUM") as midps:
            mps = [midps.tile([12, 256], F32, name=f"midacc{i}")
                   for i in range(4)]
            for h in range(P):
                nc.tensor.matmul(mps[h % 4][:],
                                 erdl[:, h * 12:h * 12 + 12],
                                 ft_sb[:, h * 256:h * 256 + 256],
                                 start=(h < 4), stop=(h >= P - 4),
                                 skip_group_check=True)
            nc.vector.tensor_copy(t0[:, 0:256], mps[0][:])
            nc.vector.tensor_tensor(t0[:, 256:512], mps[1][:], t0[:, 0:256],
                                    op=ALU.add)
            nc.vector.tensor_tensor(t0[:, 0:256], mps[2][:], t0[:, 256:512],
                                    op=ALU.add)
            nc.vector.tensor_tensor(mid_b[:], mps[3][:], t0[:, 0:256],
                                    op=ALU.add)

        # ---------------- small stage: mid^T, A'^T, G^T, u, Wc -------
        with tc.tile_pool(name="sm_ps", bufs=1, space="PSUM") as smps, \
             tc.tile_pool(name="sm_sb", bufs=1) as smsb:
            # mid^T via PE transpose of [12,128] chunks
            ps_mt = smps.tile([P, 24], BF16, tag="mt")
            for ci in range(2):
                nc.tensor.matmul(ps_mt[:, ci * 12:(ci + 1) * 12],
                                 mid_b[:, ci * P:(ci + 1) * P],
                                 ident12, is_transpose=True)
            mid_t = smsb.tile([P, 24], BF16)
            nc.vector.tensor_copy(mid_t[:], ps_mt[:])

            # A'^T = mid @ Wo2'^T   [12, 256]
            ps_at = smps.tile([12, 256], F32, tag="at")
            nc.tensor.matmul(ps_at[:], mid_t[:, 0:12], wo2_sb[:, 0:256],
                             start=True, stop=False)
            nc.tensor.matmul(ps_at[:], mid_t[:, 12:24], wo2_sb[:, 256:512],
                             start=False, stop=True)
            a_t = smsb.tile([12, 256], BF16)
            nc.vector.tensor_copy(a_t[:], ps_at[:])

            # G^T[c, o] = sum_k mid[k, c] A'^T[k, o];  chunks ci on partitions
            ps_gt = smps.tile([P, 512], F32, tag="gt")
            for ci in range(2):
                nc.tensor.matmul(ps_gt[:, ci * 256:(ci + 1) * 256],
                                 mid_b[:, ci * P:(ci + 1) * P], a_t[:],
                                 start=True, stop=True)
            gt_r = smsb.tile([P, 512], BF16)
            nc.vector.tensor_copy(gt_r[:], ps_gt[:])

            # u = G @ beta_f + beta_o  (before X so the act ops overlap X)
            for o in range(2):
                ps_u = smps.tile([P, 1], F32, tag="u")
                nc.tensor.matmul(ps_u[:], gt_r[:, o * P:(o + 1) * P],
                                 betaf_sb[:, 0:1], start=True, stop=False)
                nc.tensor.matmul(ps_u[:], gt_r[:, 256 + o * P:256 + (o + 1) * P],
                                 betaf_sb[:, 1:2], start=False, stop=True)
                nc.scalar.activation(u_sb[:, o:o + 1], ps_u[:], ACTF.Identity,
                                     bias=betao_sb[:, o:o + 1])

            # X = Wf'^T @ G^T (= (G Wf')^T); blocks a (c_in chunk) on partitions
            ps_x = smps.tile([P, 512], F32, tag="x")
            for a in range(2):
                for ci in range(2):
                    nc.tensor.matmul(ps_x[:, a * 256:(a + 1) * 256],
                                     wfn_sb[:, (ci * 2 + a) * P:(ci * 2 + a + 1) * P],
                                     gt_r[:, ci * 256:(ci + 1) * 256],
                                     start=(ci == 0), stop=(ci == 1),
                                     skip_group_check=True)
            # Wc^T = Wo1'^T + X  (blocks (a*2+o) align with [a*256 + o*128])
            for a in range(2):
                nc.vector.tensor_tensor(wc_sb[:, a * 256:(a + 1) * 256],
                                        ps_x[:, a * 256:(a + 1) * 256],
                                        wo1_sb[:, a * 256:(a + 1) * 256],
                                        op=ALU.add)

        # ---------------- pass 2: out = Wc @ f + u ----------------
        # 1024 cols per Wc-block stationary (2 back-to-back matmuls per
        # LDWEIGHTS); per-o psum pairs alternate banks; 1024-wide drains
        with tc.tile_pool(name="out_ps", bufs=2, space="PSUM") as outps, \
             tc.tile_pool(name="p2_sb", bufs=2) as p2sb:
            for tg in range(NPT // 4):
                ot0 = p2sb.tile([P, 4 * PT], BF16, tag="ot0")
                ot1 = p2sb.tile([P, 4 * PT], BF16, tag="ot1")
                for gg in range(2):
                    c0 = (tg * 4 + gg * 2) * PT
                    po0 = outps.tile([P, 2 * PT], F32, tag="po0")
                    po1 = outps.tile([P, 2 * PT], F32, tag="po1")
                    for a in range(2):
                        for o, po in ((0, po0), (1, po1)):
                            for cc in range(2):
                                nc.tensor.matmul(
                                    po[:, cc * PT:(cc + 1) * PT],
                                    wc_sb[:, (a * 2 + o) * P:(a * 2 + o + 1) * P],
                                    fn_sb[:, a * HW + c0 + cc * PT:
                                          a * HW + c0 + (cc + 1) * PT],
                                    start=(a == 0), stop=(a == 1),
                                    skip_group_check=True)
                    s0 = gg * 2 * PT
                    nc.scalar.activation(ot0[:, s0:s0 + 2 * PT], po0[:],
                                         ACTF.Identity, bias=u_sb[:, 0:1])
                    nc.vector.tensor_scalar(ot1[:, s0:s0 + 2 * PT], po1[:],
                                            u_sb[:, 1:2], None, op0=ALU.add)
                g0 = tg * 4 * PT
                if tg < NPT // 4 - 1:
                    nc.gpsimd.dma_start(out=out[:, g0:g0 + 4 * PT], in_=ot0[:])
                    nc.gpsimd.dma_start(out=out[:, HW + g0:HW + g0 + 4 * PT],
                                        in_=ot1[:])
                else:
                    # split the last group's stores so the tail is short
                    for hh in range(2):
                        s0 = hh * 2 * PT
                        nc.gpsimd.dma_start(
                            out=out[:, g0 + s0:g0 + s0 + 2 * PT],
                            in_=ot0[:, s0:s0 + 2 * PT])
                        nc.gpsimd.dma_start(
                            out=out[:, HW + g0 + s0:HW + g0 + s0 + 2 * PT],
                            in_=ot1[:, s0:s0 + 2 * PT])

    nc.compile()
    _NC_CACHE["nc"] = nc
    return nc


def _band_consts():
    idx = np.arange(P)
    # erosion: output i covers input [i-8, i+4]; dilation: [i-4, i+8]
    band_er = ((idx[:, None] >= idx[None, :] - 8) &
               (idx[:, None] <= idx[None, :] + 4)).astype(np.float32)
    band_dl = ((idx[:, None] >= idx[None, :] - 4) &
               (idx[:, None] <= idx[None, :] + 8)).astype(np.float32)
    return band_er, band_dl


def _host_masks(mb):
    """fbu channels for one batch, transposed: [w, k*128+h] with k = 3n
    (er_n), 3n+1 (bg_n), 3n+2 (maskd_n) -- all 0/1, exact in bf16.

    Exact integer morphology via the band-matmul method (composition of 4
    cv2-style 4x4 erode/dilate iterations = 13-tap separable min/max)."""
    band_er, band_dl = _band_consts()
    cnt = band_er.sum(axis=0)                      # [128] per output index
    z = (mb > 0.3).astype(np.float32)              # [N, H, W]
    # rows pass: out[i, n, w] = sum_h band[h, i] z[n, h, w]
    r_er = (np.einsum('hi,nhw->inw', band_er, z)
            == cnt[:, None, None]).astype(np.float32)
    r_dl = (np.einsum('hi,nhw->inw', band_dl, z) > 0.5).astype(np.float32)
    # cols pass: er_t[iw, n, ih] = sum_w band[w, iw] rows[ih, n, w]
    er_t = np.einsum('wi,hnw->inh', band_er, r_er) == cnt[:, None, None]
    dl_t = np.einsum('wi,hnw->inh', band_dl, r_dl) > 0.5
    fbu = np.empty((P, P, 12), np.float32)         # [w, h, k]
    for n in range(N):
        fbu[:, :, 3 * n] = er_t[:, n, :]
        fbu[:, :, 3 * n + 1] = ~dl_t[:, n, :]
        fbu[:, :, 3 * n + 2] = dl_t[:, n, :] & ~er_t[:, n, :]
    return fbu.reshape(P, 12 * P).astype(ml_dtypes.bfloat16)


def prepare_in_maps(feature, m, W_f, g_f, b_f, mu_f, v_f, W_o, g_o, b_o, mu_o, v_o):
    feature = np.asarray(feature, dtype=np.float32)
    m = np.asarray(m, dtype=np.float32)
    W_f = np.asarray(W_f, dtype=np.float32)
    W_o = np.asarray(W_o, dtype=np.float32)
    g_f, b_f, mu_f, v_f = (np.asarray(x, dtype=np.float32) for x in (g_f, b_f, mu_f, v_f))
    g_o, b_o, mu_o, v_o = (np.asarray(x, dtype=np.float32) for x in (g_o, b_o, mu_o, v_o))

    inv_f = g_f / np.sqrt(v_f + EPS)
    beta_f_v = b_f - mu_f * inv_f
    inv_o = g_o / np.sqrt(v_o + EPS)
    beta_o_v = b_o - mu_o * inv_o
    Wf_p = (inv_f[:, None] * W_f).astype(np.float32)          # [C, C]
    Wo1_p = (inv_o[:, None] * W_o[:, :C]).astype(np.float32)  # [C, C]
    Wo2_p = (inv_o[:, None] * W_o[:, C:]).astype(np.float32)  # [C, C]

    def blocks_t(Wp):
        # lhsT layout: blocks ci*2+o of Wp^T
        a = np.empty((P, 512), np.float32)
        for ci in range(2):
            for o in range(2):
                a[:, (ci * 2 + o) * P:(ci * 2 + o + 1) * P] = \
                    Wp[o * P:(o + 1) * P, ci * P:(ci + 1) * P].T
        return a

    def blocks_n(Wp):
        # natural-layout blocks ci*2+a: Wp[ci*128:(ci+1)*128, a*128:(a+1)*128]
        a_ = np.empty((P, 512), np.float32)
        for ci in range(2):
            for a in range(2):
                a_[:, (ci * 2 + a) * P:(ci * 2 + a + 1) * P] = \
                    Wp[ci * P:(ci + 1) * P, a * P:(a + 1) * P]
        return a_

    bf = ml_dtypes.bfloat16

    pkb = np.zeros((P, PKB_W), bf)
    pkb[:, 0:512] = np.concatenate([Wo2_p.T[0:P, :], Wo2_p.T[P:C, :]],
                                   axis=1).astype(bf)
    pkb[:, 512:1024] = blocks_n(Wf_p).astype(bf)
    pkb[:, 1024:1026] = beta_f_v.reshape(2, P).T.astype(bf)
    pkb[0:12, 1026:1038] = np.eye(12, dtype=np.float32).astype(bf)

    pkf = np.zeros((P, PKF_W), np.float32)
    pkf[:, 0:512] = blocks_t(Wo1_p)
    pkf[:, 512:514] = beta_o_v.reshape(2, P).T

    in_maps = []
    for b in range(B):
        im = {"pkb": pkb, "pkf": pkf}
        im["erdl"] = _host_masks(m[b])
        f16 = feature[b].reshape(C, HW).astype(bf)
        im["featn"] = np.ascontiguousarray(
            np.concatenate([f16[0:P, :], f16[P:C, :]], axis=1))
        # featt[w, h*256 + c] = f[c, h*128 + w]
        im["featt"] = np.ascontiguousarray(
            f16.reshape(C, P, P).transpose(2, 1, 0).reshape(P, 2 * HW))
        in_maps.append(im)
    return in_maps


def kernel(feature, m, W_f, g_f, b_f, mu_f, v_f, W_o, g_o, b_o, mu_o, v_o):
    nc = build()
    in_maps = prepare_in_maps(feature, m, W_f, g_f, b_f, mu_f, v_f,
                              W_o, g_o, b_o, mu_o, v_o)
    res = bass_utils.run_bass_kernel_spmd(nc, in_maps, list(range(B)))
    out = np.empty((B, C, H, W), np.float32)
    for b in range(B):
        o = np.asarray(res.results[b]["out"]).astype(np.float32)
        out[b, 0:P] = o[:, 0:HW].reshape(P, H, W)
        out[b, P:C] = o[:, HW:2 * HW].reshape(P, H, W)
    return out
